# revision 1
# baseline (speedup 1.0000x reference)
"""Local (sliding-window) attention kernel for Trainium2, 8 NeuronCores.

Problem: x [B=2, L=2048, E=512] fp32; q/k/v = x @ W{q,k,v}.T + b; scores over a
+-64 window, softmax, out = probs @ v_win.

Sharding: 8 cores = (batch 2) x (4 sequence chunks of 512 queries). Each core
gets a transposed, halo'd slice xT [E, 640] (64 halo keys each side,
zero-padded at sequence ends) and computes its own q/k/v projections
(weights replicated), then 4 blocks of 128 queries x 256-key-span windowed
attention. Matmul inputs are fp16 (PSUM accumulates fp32): full PE rate, and
~8x tighter rounding than bf16. Softmax skips max-subtraction (scores are
O(1): x~N(0,1), W~0.02 scale) and masks multiplicatively after exp.
"""

import numpy as np

B, L, E = 2, 2048, 512
WHALF = 64
NCORES = 8
CHUNK = 512            # queries per core
SPAN = CHUNK + 2 * WHALF   # 640 key/value positions per core
BLK = 128              # query block
NBLK = CHUNK // BLK    # 4
KSPAN = 2 * BLK        # 256-key span per query block
EC = E // 128          # 4 e-chunks

_CACHE = {}


def _build_bass():
    import concourse.bass as bass
    import concourse.mybir as mybir
    from concourse.tile import TileContext

    f32 = mybir.dt.float32
    f16 = mybir.dt.float16
    AF = mybir.ActivationFunctionType
    AX = mybir.AxisListType

    nc = bass.Bass()
    xT = nc.dram_tensor("xT", [E, SPAN], f16, kind="ExternalInput")
    wqT = nc.dram_tensor("wqT", [E, E], f16, kind="ExternalInput")
    wkT = nc.dram_tensor("wkT", [E, E], f16, kind="ExternalInput")
    wvT = nc.dram_tensor("wvT", [E, E], f16, kind="ExternalInput")
    bqk = nc.dram_tensor("bqk", [E, 2], f32, kind="ExternalInput")
    masks = nc.dram_tensor("masks", [NBLK, BLK, KSPAN], f16, kind="ExternalInput")
    ident = nc.dram_tensor("ident", [BLK, BLK], f16, kind="ExternalInput")
    out = nc.dram_tensor("out", [CHUNK, E], f32, kind="ExternalOutput")
    # Dummy output that keeps the PE warm-up matmul stream live (not read by
    # the host). HAM throttles TensorE to 1.2 GHz until ~3.4us of sustained
    # activity; the warm-up stream spans the DMA-bound kernel start so real
    # matmuls begin at 2.4 GHz.
    warm_out = nc.dram_tensor("warm_out", [128, 16], f32, kind="ExternalOutput")

    with TileContext(nc) as tc:
        with tc.tile_pool(name="sb", bufs=1) as sb, \
             tc.tile_pool(name="ps", bufs=5, space="PSUM") as ps, \
             tc.tile_pool(name="ps2", bufs=2, space="PSUM") as ps2:
            # ---------- input DMAs (consumption order) ----------
            # Sync (HWDGE) issues the early-needed tensors; GpSimd (SWDGE)
            # issues the rest in parallel — DMA issue costs ~650ns each and
            # serializes per issuing engine.
            xt = sb.tile([128, EC, SPAN], f16)
            wq = sb.tile([128, EC, E], f16)
            wk = sb.tile([128, EC, E], f16)
            wv = sb.tile([128, EC, E], f16)
            nc.sync.dma_start(out=wq[:, 0, :], in_=wqT[0:128, :])
            nc.sync.dma_start(out=xt[:, 0, :], in_=xT[0:128, :])
            for c in range(1, EC):
                nc.sync.dma_start(out=wq[:, c, :], in_=wqT[c * 128:(c + 1) * 128, :])
                nc.sync.dma_start(out=xt[:, c, :], in_=xT[c * 128:(c + 1) * 128, :])
            bqk_t = sb.tile([128, EC, 2], f32)
            nc.sync.dma_start(out=bqk_t[:], in_=bqk.rearrange("(c p) t -> p c t", p=128))
            nc.gpsimd.dma_start(out=wk[:], in_=wkT.rearrange("(c p) e -> p c e", p=128))
            nc.gpsimd.dma_start(out=wv[:], in_=wvT.rearrange("(c p) e -> p c e", p=128))
            idt = sb.tile([128, BLK], f16)
            nc.gpsimd.dma_start(out=idt[:], in_=ident[:])
            msk = sb.tile([128, NBLK, KSPAN], f16)
            nc.gpsimd.dma_start(out=msk[:], in_=masks.rearrange("n p k -> p n k"))

            # ---------- PE warm-up stream (no input deps: memset + matmuls)
            wrm = sb.tile([128, E], f16)
            nc.vector.memset(wrm[:], 0.0)
            w_ps = ps2.tile([128, E], f32, tag="warm", bufs=1)
            for _ in range(34):
                nc.tensor.matmul(w_ps[:], wrm[:, 0:128], wrm[:],
                                 start=True, stop=True)
            w_sb = sb.tile([128, 16], f32)
            nc.vector.tensor_copy(w_sb[:], w_ps[:, 0:16])
            nc.sync.dma_start(out=warm_out[:], in_=w_sb[:])

            # ---------- q projection: qT [e_out, l] fp16 ----------
            # ec-outer over 4 concurrent PSUM groups so the first matmuls only
            # need chunk-0 DMAs.
            qt = sb.tile([128, EC, CHUNK], f16)
            q_ps = [ps.tile([128, CHUNK], f32, tag="mm", name=f"qps{fc}")
                    for fc in range(EC)]
            for ec in range(EC):
                for fc in range(EC):
                    nc.tensor.matmul(
                        q_ps[fc][:],
                        wq[:, ec, fc * 128:(fc + 1) * 128],
                        xt[:, ec, WHALF:WHALF + CHUNK],
                        start=(ec == 0), stop=(ec == EC - 1))
            for fc in range(EC):
                nc.scalar.activation(qt[:, fc, :], q_ps[fc][:], AF.Identity,
                                     bias=bqk_t[:, fc, 0:1])

            # ---------- k projection: kT [e_out, j] over full 640 span ----------
            # split 640 = 2 x 320 (psum bank limit; fp32r-free fp16 path)
            kt = sb.tile([128, EC, SPAN], f16)
            for half in range(2):
                j0 = half * 320
                k_ps = [ps.tile([128, 320], f32, tag="mm", name=f"kps{half}_{fc}")
                        for fc in range(EC)]
                for ec in range(EC):
                    for fc in range(EC):
                        nc.tensor.matmul(
                            k_ps[fc][:],
                            wk[:, ec, fc * 128:(fc + 1) * 128],
                            xt[:, ec, j0:j0 + 320],
                            start=(ec == 0), stop=(ec == EC - 1))
                for fc in range(EC):
                    nc.scalar.activation(kt[:, fc, j0:j0 + 320], k_ps[fc][:],
                                         AF.Identity, bias=bqk_t[:, fc, 1:2])

            # ---------- v projection: natural [j, f] layout ----------
            v_sb = sb.tile([128, SPAN // 128, E], f16)
            for wave in ([0, 1, 2, 3], [4]):
                v_ps = {jc: ps.tile([128, E], f32, tag="mm", name=f"vps{jc}")
                        for jc in wave}
                for ec in range(EC):
                    for jc in wave:
                        nc.tensor.matmul(
                            v_ps[jc][:],
                            xt[:, ec, jc * 128:(jc + 1) * 128],
                            wv[:, ec, :],
                            start=(ec == 0), stop=(ec == EC - 1))
                for jc in wave:
                    nc.vector.tensor_copy(v_sb[:, jc, :], v_ps[jc][:])

            # ---------- windowed attention, 4 blocks of 128 queries ----------
            # Software-pipelined emission: block i+1's score matmuls are
            # emitted before block i's transposes/AV so the in-order PE stream
            # has independent work while block i's softmax runs on ACT/DVE.
            inv_sqrt_e = float(1.0 / np.sqrt(E))

            def emit_scores(i):
                s_ps = ps.tile([128, KSPAN], f32, tag="mm", name=f"sps{i}")
                for ec in range(EC):
                    nc.tensor.matmul(
                        s_ps[:],
                        qt[:, ec, i * BLK:(i + 1) * BLK],
                        kt[:, ec, i * BLK:i * BLK + KSPAN],
                        start=(ec == 0), stop=(ec == EC - 1))
                return s_ps

            s_tiles = {0: emit_scores(0)}
            for i in range(NBLK):
                s_ps = s_tiles.pop(i)
                e_sb = sb.tile([128, KSPAN], f16, tag="esb", name=f"esb{i}", bufs=3)
                nc.scalar.activation(e_sb[:], s_ps[:], AF.Exp, scale=inv_sqrt_e)
                nc.vector.tensor_mul(e_sb[:], e_sb[:], msk[:, i, :])
                r = sb.tile([128, 1], f32, tag="r", name=f"r{i}", bufs=2)
                nc.vector.reduce_sum(out=r[:], in_=e_sb[:], axis=AX.X)
                rinv = sb.tile([128, 1], f32, tag="rinv", name=f"rinv{i}", bufs=2)
                nc.vector.reciprocal(rinv[:], r[:])
                if i + 1 < NBLK:
                    s_tiles[i + 1] = emit_scores(i + 1)
                pt_ps = ps2.tile([128, 2, BLK], f16, tag="pt", name=f"ptps{i}")
                nc.tensor.transpose(pt_ps[:, 0, :], e_sb[:, 0:BLK], idt[:])
                nc.tensor.transpose(pt_ps[:, 1, :], e_sb[:, BLK:KSPAN], idt[:])
                pt_sb = sb.tile([128, 2, BLK], f16, tag="ptsb", name=f"ptsb{i}", bufs=3)
                nc.vector.tensor_copy(pt_sb[:, 0, :], pt_ps[:, 0, :])
                nc.vector.tensor_copy(pt_sb[:, 1, :], pt_ps[:, 1, :])
                o_ps = ps.tile([128, E], f32, tag="mm", name=f"ops{i}")
                nc.tensor.matmul(o_ps[:], pt_sb[:, 0, :], v_sb[:, i, :],
                                 start=True, stop=False)
                nc.tensor.matmul(o_ps[:], pt_sb[:, 1, :], v_sb[:, i + 1, :],
                                 start=False, stop=True)
                o_sb = sb.tile([128, E], f32, tag="osb", name=f"osb{i}", bufs=3)
                nc.scalar.activation(o_sb[:], o_ps[:], AF.Copy, scale=rinv[:])
                nc.sync.dma_start(out=out[i * BLK:(i + 1) * BLK, :], in_=o_sb[:])

    _split_multi_waits(nc)
    return nc


def _split_multi_waits(nc):
    """This walrus build accepts only ONE sync wait per engine instruction;
    Tile emits 2+ on phase-crossing instructions. Peel extra waits onto
    same-engine NoOps placed immediately before (engine streams are in-order,
    so the waits still guard the instruction)."""
    import concourse.mybir as mybir

    for fn in nc.m.functions:
        for blk in fn.blocks:
            new_insts = []
            for inst in blk.instructions:
                si = inst.sync_info
                waits = list(si.on_wait) if si is not None and si.on_wait else []
                if len(waits) > 1:
                    for w in waits[:-1]:
                        new_insts.append(mybir.InstNoOp(
                            name=nc.get_next_instruction_name(),
                            engine=inst.engine,
                            ins=[], outs=[],
                            sync_info=mybir.SyncInfo(on_wait=[w], on_update=[]),
                        ))
                    inst.sync_info = mybir.SyncInfo(
                        on_wait=[waits[-1]], on_update=list(si.on_update or []))
                new_insts.append(inst)
            blk.instructions = new_insts


def _host_inputs(x, Wq, bq, Wk, bk, Wv, bv):
    wq16 = np.ascontiguousarray(Wq.T).astype(np.float16)
    wk16 = np.ascontiguousarray(Wk.T).astype(np.float16)
    wv16 = np.ascontiguousarray(Wv.T).astype(np.float16)
    bqk = np.stack([bq, bk], axis=1).astype(np.float32)  # [E, 2]
    idn = np.eye(BLK, dtype=np.float16)
    p = np.arange(BLK)[:, None]
    jj = np.arange(KSPAN)[None, :]
    band = (jj >= p) & (jj <= p + 2 * WHALF)
    in_maps = []
    for c in range(NCORES):
        b, ci = divmod(c, NBLK)
        s = ci * CHUNK
        lo, hi = s - WHALF, s + CHUNK + WHALF
        a0, a1 = max(lo, 0), min(hi, L)
        xh = np.zeros((SPAN, E), np.float32)
        xh[a0 - lo:a1 - lo] = x[b, a0:a1]
        m = np.zeros((NBLK, BLK, KSPAN), np.float16)
        for i in range(NBLK):
            g = s - WHALF + i * BLK + jj  # global key index [1, KSPAN]
            m[i] = (band & (g >= 0) & (g < L)).astype(np.float16)
        in_maps.append({
            "xT": np.ascontiguousarray(xh.T).astype(np.float16),
            "wqT": wq16, "wkT": wk16, "wvT": wv16,
            "bqk": bqk, "masks": m, "ident": idn,
        })
    return in_maps


def kernel(x, Wq, bq, Wk, bk, Wv, bv, window_size, _trace=False):
    from concourse import bass_utils

    x = np.asarray(x, dtype=np.float32)
    Wq = np.asarray(Wq, dtype=np.float32)
    Wk = np.asarray(Wk, dtype=np.float32)
    Wv = np.asarray(Wv, dtype=np.float32)
    bq = np.asarray(bq, dtype=np.float32)
    bk = np.asarray(bk, dtype=np.float32)
    bv = np.asarray(bv, dtype=np.float32)
    assert int(window_size) == WHALF, f"kernel hardcodes window_size={WHALF}"
    assert x.shape == (B, L, E)

    if "nc" not in _CACHE:
        _CACHE["nc"] = _build_bass()
    nc = _CACHE["nc"]

    in_maps = _host_inputs(x, Wq, bq, Wk, bk, Wv, bv)
    res = bass_utils.run_bass_kernel_spmd(
        nc, in_maps, core_ids=list(range(NCORES)), trace=_trace)
    _CACHE["last_results"] = res

    out = np.empty((B, L, E), np.float32)
    for c in range(NCORES):
        b, ci = divmod(c, NBLK)
        out[b, ci * CHUNK:(ci + 1) * CHUNK] = res.results[c]["out"]
    if np.any(bv):
        out += bv[None, None, :]  # sum(probs) == 1 makes the v-bias additive
    return out



# revision 2
# speedup vs baseline: 1.0558x; 1.0558x over previous
"""Local (sliding-window) attention kernel for Trainium2, 8 NeuronCores.

Problem: x [B=2, L=2048, E=512] fp32; q/k/v = x @ W{q,k,v}.T + b; scores over a
+-64 window, softmax, out = probs @ v_win.

Sharding: 8 cores = (batch 2) x (4 sequence chunks of 512 queries). Each core
gets a transposed, halo'd slice xT [E, 640] (64 halo keys each side,
zero-padded at sequence ends) and computes its own q/k/v projections
(weights replicated), then 4 blocks of 128 queries x 256-key-span windowed
attention. Matmul inputs are fp16 (PSUM accumulates fp32).

Schedule (PE-order): short warm-up (HAM clock ramp) -> q proj (ec-outer, gated
only on chunk-0 DMAs) -> k proj -> v proj -> scores (all 4 blocks) ->
transpose+AV per block. Softmax (exp on ACT with fused row-sum accum_out) runs
on Scalar/Vector while the PE streams v and later blocks' scores, so its
latency is hidden. The window mask is folded into the scores matmul as an
additive -1e4 term (extra accumulation matmul vs the identity), which keeps
the post-exp row-sum correct without a separate DVE mask multiply. Outputs are
scaled by 1/rowsum on DVE and DMA'd out in fp16 (host upcasts).

DMA plan: inputs are issued on both queues (Sync: xt chunks + wv + bqk;
GpSimd: wq chunks, wk, masks+ident blob) in first-need order so the first q
matmul is gated only on xt_c0/wq_c0 and everything else lands ahead of use.
"""

import numpy as np

B, L, E = 2, 2048, 512
WHALF = 64
NCORES = 8
CHUNK = 512            # queries per core
SPAN = CHUNK + 2 * WHALF   # 640 key/value positions per core
BLK = 128              # query block
NBLK = CHUNK // BLK    # 4
KSPAN = 2 * BLK        # 256-key span per query block
EC = E // 128          # 4 e-chunks
N_WARM = 3             # warm-up matmuls (HAM ramp + cover DMA latency)
MASK_NEG = -10000.0    # additive mask value (pre exp-scale)

_CACHE = {}


def _build_bass():
    import concourse.bass as bass
    import concourse.mybir as mybir
    from concourse.tile import TileContext

    f32 = mybir.dt.float32
    f16 = mybir.dt.float16
    AF = mybir.ActivationFunctionType

    nc = bass.Bass()
    xT = nc.dram_tensor("xT", [E, SPAN], f16, kind="ExternalInput")
    wqT = nc.dram_tensor("wqT", [E, E], f16, kind="ExternalInput")
    wkT = nc.dram_tensor("wkT", [E, E], f16, kind="ExternalInput")
    wvT = nc.dram_tensor("wvT", [E, E], f16, kind="ExternalInput")
    bqk = nc.dram_tensor("bqk", [128, 2 * EC], f32, kind="ExternalInput")
    # masks (additive, 0 / -1e4) for the 4 blocks + 128x128 identity, packed.
    mi = nc.dram_tensor("mi", [128, NBLK * KSPAN + BLK], f16,
                        kind="ExternalInput")
    out = nc.dram_tensor("out", [CHUNK, E], f16, kind="ExternalOutput")
    # Dummy output that keeps the PE warm-up matmul stream live (not read by
    # the host). HAM throttles TensorE until a few us of sustained activity;
    # the warm-up bridges the gap until the first input DMAs land.
    warm_out = nc.dram_tensor("warm_out", [128, 16], f32, kind="ExternalOutput")

    inv_sqrt_e = float(1.0 / np.sqrt(E))

    with TileContext(nc) as tc:
        with tc.tile_pool(name="sb", bufs=1) as sb, \
             tc.tile_pool(name="ps", bufs=4, space="PSUM") as ps, \
             tc.tile_pool(name="pss", bufs=3, space="PSUM") as pss, \
             tc.tile_pool(name="psp", bufs=1, space="PSUM") as psp:
            # ---------- input DMAs, two queues, first-need order ----------
            xt = sb.tile([128, EC, SPAN], f16)
            wq = sb.tile([128, EC, E], f16)
            wk = sb.tile([128, EC, E], f16)
            wv = sb.tile([128, EC, E], f16)
            bqk_t = sb.tile([128, 2 * EC], f32)
            mi_t = sb.tile([128, NBLK * KSPAN + BLK], f16)

            # Sync (HWDGE): xt chunks 0..3, wv, bqk
            for c in range(EC):
                nc.sync.dma_start(out=xt[:, c, :], in_=xT[c * 128:(c + 1) * 128, :])
            nc.sync.dma_start(out=wv[:], in_=wvT.rearrange("(c p) e -> p c e", p=128))
            nc.sync.dma_start(out=bqk_t[:], in_=bqk[:])
            # GpSimd (SWDGE): wq c0, wq c1-3, wk, masks+ident
            nc.gpsimd.dma_start(out=wq[:, 0, :], in_=wqT[0:128, :])
            nc.gpsimd.dma_start(
                out=wq[:, 1:EC, :],
                in_=wqT[128:E, :].rearrange("(c p) e -> p c e", p=128))
            nc.gpsimd.dma_start(out=wk[:], in_=wkT.rearrange("(c p) e -> p c e", p=128))
            nc.gpsimd.dma_start(out=mi_t[:], in_=mi[:])

            def msk(i):
                return mi_t[:, i * KSPAN:(i + 1) * KSPAN]
            idt = mi_t[:, NBLK * KSPAN:NBLK * KSPAN + BLK]

            # ---------- PE warm-up stream (no input deps) ----------
            wrm = sb.tile([128, E], f16)
            nc.vector.memset(wrm[:], 0.0)
            w_ps = ps.tile([128, E], f32, tag="mm", name="warm")
            for _ in range(N_WARM):
                nc.tensor.matmul(w_ps[:], wrm[:, 0:128], wrm[:],
                                 start=True, stop=True)
            w_sb = sb.tile([128, 16], f32)
            nc.vector.tensor_copy(w_sb[:], w_ps[:, 0:16])
            nc.gpsimd.dma_start(out=warm_out[:], in_=w_sb[:])

            # ---------- q projection: qT [e_out, l] fp16 ----------
            # ec-outer over 4 concurrent PSUM groups so the first matmuls only
            # need chunk-0 DMAs. Bias+copy to SBUF on Scalar (ACT).
            qt = sb.tile([128, EC, CHUNK], f16)
            q_ps = [ps.tile([128, CHUNK], f32, tag="mm", name=f"qps{fc}")
                    for fc in range(EC)]
            for ec in range(EC):
                for fc in range(EC):
                    nc.tensor.matmul(
                        q_ps[fc][:],
                        wq[:, ec, fc * 128:(fc + 1) * 128],
                        xt[:, ec, WHALF:WHALF + CHUNK],
                        start=(ec == 0), stop=(ec == EC - 1))
            for fc in range(EC):
                nc.scalar.activation(qt[:, fc, :], q_ps[fc][:], AF.Identity,
                                     bias=bqk_t[:, 2 * fc:2 * fc + 1])

            # ---------- k projection: kT [e_out, j] over full 640 span ----------
            # split 640 = 2 x 320 (psum bank limit). Bias+copy on Vector (DVE)
            # so Scalar stays free for the q ACTs and the exps.
            kt = sb.tile([128, EC, SPAN], f16)
            for half in range(2):
                j0 = half * 320
                k_ps = [ps.tile([128, 320], f32, tag="mm", name=f"kps{half}_{fc}")
                        for fc in range(EC)]
                for ec in range(EC):
                    for fc in range(EC):
                        nc.tensor.matmul(
                            k_ps[fc][:],
                            wk[:, ec, fc * 128:(fc + 1) * 128],
                            xt[:, ec, j0:j0 + 320],
                            start=(ec == 0), stop=(ec == EC - 1))
                for fc in range(EC):
                    nc.vector.tensor_scalar_add(
                        kt[:, fc, j0:j0 + 320], k_ps[fc][:],
                        bqk_t[:, 2 * fc + 1:2 * fc + 2])

            # ---------- v projection: natural [j, f] layout ----------
            v_sb = sb.tile([128, SPAN // 128, E], f16)
            for wave in ([0, 1, 2, 3], [4]):
                v_ps = {jc: ps.tile([128, E], f32, tag="mm", name=f"vps{jc}")
                        for jc in wave}
                for ec in range(EC):
                    for jc in wave:
                        nc.tensor.matmul(
                            v_ps[jc][:],
                            xt[:, ec, jc * 128:(jc + 1) * 128],
                            wv[:, ec, :],
                            start=(ec == 0), stop=(ec == EC - 1))
                for jc in wave:
                    nc.vector.tensor_copy(v_sb[:, jc, :], v_ps[jc][:])

            # ---------- scores for all 4 blocks ----------
            # s = mask + sum_ec qT.T @ kT ; mask lands via identity matmul as
            # the first accumulation step (idt.T @ msk == msk).
            s_tiles = []
            for i in range(NBLK):
                s_ps = pss.tile([128, KSPAN], f32, tag="s", name=f"sps{i}")
                nc.tensor.matmul(s_ps[:], idt, msk(i), start=True, stop=False)
                for ec in range(EC):
                    nc.tensor.matmul(
                        s_ps[:],
                        qt[:, ec, i * BLK:(i + 1) * BLK],
                        kt[:, ec, i * BLK:i * BLK + KSPAN],
                        start=False, stop=(ec == EC - 1))
                s_tiles.append(s_ps)

            # softmax numerator + row-sum on Scalar (exp w/ fused accum_out);
            # skipping max-subtraction (scores are O(1): x~N(0,1), W~0.02).
            e_sbs, rinvs = [], []
            for i in range(NBLK):
                e_sb = sb.tile([128, KSPAN], f16, tag="esb", name=f"esb{i}", bufs=4)
                r = sb.tile([128, 1], f32, tag="r", name=f"r{i}", bufs=4)
                nc.scalar.activation(e_sb[:], s_tiles[i][:], AF.Exp,
                                     scale=inv_sqrt_e, accum_out=r[:])
                rinv = sb.tile([128, 1], f32, tag="rinv", name=f"rinv{i}", bufs=4)
                nc.vector.reciprocal(rinv[:], r[:])
                e_sbs.append(e_sb)
                rinvs.append(rinv)

            # ---------- transpose probs + AV per block ----------
            for i in range(NBLK):
                pt_ps = psp.tile([128, 2, BLK], f16, tag="pt", name=f"ptps{i}")
                nc.tensor.transpose(pt_ps[:, 0, :], e_sbs[i][:, 0:BLK], idt)
                nc.tensor.transpose(pt_ps[:, 1, :], e_sbs[i][:, BLK:KSPAN], idt)
                pt_sb = sb.tile([128, 2, BLK], f16, tag="ptsb", name=f"ptsb{i}", bufs=2)
                nc.vector.tensor_copy(pt_sb[:, 0, :], pt_ps[:, 0, :])
                nc.vector.tensor_copy(pt_sb[:, 1, :], pt_ps[:, 1, :])
                o_ps = ps.tile([128, E], f32, tag="mm", name=f"ops{i}")
                nc.tensor.matmul(o_ps[:], pt_sb[:, 0, :], v_sb[:, i, :],
                                 start=True, stop=False)
                nc.tensor.matmul(o_ps[:], pt_sb[:, 1, :], v_sb[:, i + 1, :],
                                 start=False, stop=True)
                o_sb = sb.tile([128, E], f16, tag="osb", name=f"osb{i}", bufs=2)
                nc.vector.tensor_scalar_mul(o_sb[:], o_ps[:], rinvs[i][:])
                nc.sync.dma_start(out=out[i * BLK:(i + 1) * BLK, :], in_=o_sb[:])

    _split_multi_waits(nc)
    return nc


def _split_multi_waits(nc):
    """This walrus build accepts only ONE sync wait per engine instruction;
    Tile emits 2+ on phase-crossing instructions. Peel extra waits onto
    same-engine NoOps placed immediately before (engine streams are in-order,
    so the waits still guard the instruction)."""
    import concourse.mybir as mybir

    for fn in nc.m.functions:
        for blk in fn.blocks:
            new_insts = []
            for inst in blk.instructions:
                si = inst.sync_info
                waits = list(si.on_wait) if si is not None and si.on_wait else []
                if len(waits) > 1:
                    for w in waits[:-1]:
                        new_insts.append(mybir.InstNoOp(
                            name=nc.get_next_instruction_name(),
                            engine=inst.engine,
                            ins=[], outs=[],
                            sync_info=mybir.SyncInfo(on_wait=[w], on_update=[]),
                        ))
                    inst.sync_info = mybir.SyncInfo(
                        on_wait=[waits[-1]], on_update=list(si.on_update or []))
                new_insts.append(inst)
            blk.instructions = new_insts


def _host_inputs(x, Wq, bq, Wk, bk, Wv, bv):
    wq16 = np.ascontiguousarray(Wq.T).astype(np.float16)
    wk16 = np.ascontiguousarray(Wk.T).astype(np.float16)
    wv16 = np.ascontiguousarray(Wv.T).astype(np.float16)
    # bias pairs per partition row: [p, 2*c + {0:q,1:k}]
    bqk = np.stack([bq.reshape(EC, 128).T, bk.reshape(EC, 128).T],
                   axis=-1).reshape(128, 2 * EC).astype(np.float32)
    idn = np.eye(BLK, dtype=np.float16)
    p = np.arange(BLK)[:, None]
    jj = np.arange(KSPAN)[None, :]
    band = (jj >= p) & (jj <= p + 2 * WHALF)
    in_maps = []
    for c in range(NCORES):
        b, ci = divmod(c, NBLK)
        s = ci * CHUNK
        lo, hi = s - WHALF, s + CHUNK + WHALF
        a0, a1 = max(lo, 0), min(hi, L)
        xh = np.zeros((SPAN, E), np.float32)
        xh[a0 - lo:a1 - lo] = x[b, a0:a1]
        m = np.full((NBLK, BLK, KSPAN), MASK_NEG, np.float16)
        for i in range(NBLK):
            g = s - WHALF + i * BLK + jj  # global key index [1, KSPAN]
            m[i][band & (g >= 0) & (g < L)] = 0.0
        mi = np.concatenate(
            [m.transpose(1, 0, 2).reshape(BLK, NBLK * KSPAN), idn], axis=1)
        in_maps.append({
            "xT": np.ascontiguousarray(xh.T).astype(np.float16),
            "wqT": wq16, "wkT": wk16, "wvT": wv16,
            "bqk": bqk, "mi": np.ascontiguousarray(mi),
        })
    return in_maps


def kernel(x, Wq, bq, Wk, bk, Wv, bv, window_size, _trace=False):
    from concourse import bass_utils

    x = np.asarray(x, dtype=np.float32)
    Wq = np.asarray(Wq, dtype=np.float32)
    Wk = np.asarray(Wk, dtype=np.float32)
    Wv = np.asarray(Wv, dtype=np.float32)
    bq = np.asarray(bq, dtype=np.float32)
    bk = np.asarray(bk, dtype=np.float32)
    bv = np.asarray(bv, dtype=np.float32)
    assert int(window_size) == WHALF, f"kernel hardcodes window_size={WHALF}"
    assert x.shape == (B, L, E)

    if "nc" not in _CACHE:
        _CACHE["nc"] = _build_bass()
    nc = _CACHE["nc"]

    in_maps = _host_inputs(x, Wq, bq, Wk, bk, Wv, bv)
    res = bass_utils.run_bass_kernel_spmd(
        nc, in_maps, core_ids=list(range(NCORES)), trace=_trace)
    _CACHE["last_results"] = res

    out = np.empty((B, L, E), np.float32)
    for c in range(NCORES):
        b, ci = divmod(c, NBLK)
        out[b, ci * CHUNK:(ci + 1) * CHUNK] = res.results[c]["out"].astype(np.float32)
    if np.any(bv):
        out += bv[None, None, :]  # sum(probs) == 1 makes the v-bias additive
    return out
